# revision 2
# baseline (speedup 1.0000x reference)
"""AttnPool3D Trainium2 kernel — transposed (spatial-on-partitions) design.

Reference computation (B=2, C=128, D=48, H=96, W=96, N = D*H*W = 442368):
    logits = einsum('bcdhw,c->bdhw', feat, w_attn) + 2.0*clip(mask, 0, 1)
    w = softmax(logits.reshape(B, -1), axis=-1)
    out = einsum('bcn,bn->bc', feat.reshape(B, C, -1), w)

Sharding: 8 cores = (batch b in 0..1) x (spatial quarter q in 0..3).
Each core processes NS = 110592 spatial positions in one pass.

Layout: feat is shipped TRANSPOSED as fp16: featT[p, t*128 + c] =
feat[c, t*128 + p] — i.e. 864 tiles of [128 spatial positions x 128
channels].  With spatial on partitions:
  - logits: one DVE scalar_tensor_tensor per tile, (fT * 1) * w_bcast with
    fused accum_out -> logit column [128, 1].  (contraction over C = free dim)
  - mask add (incl. the exp bias -8): per-partition values now — one cheap
    [128, CH] DVE add per chunk (mask pre-scaled/biased on host, fp32).
  - exp: ACT on [128, CH] blocks (~1k cols total vs 110k in the broadcast
    design), accum_out -> per-chunk sumexp column.
  - pass 2 (v[c] = sum_n p[n] f[c,n]): per tile one PE matmul with the
    fp16 p-column STATIONARY and the fT tile MOVING -> out [1, 128c]
    accumulated in PSUM across all 864 tiles.  PE does ONE pass over the
    data (~46 us) vs 3 passes (~138 us) in the broadcast design.
Engine budget per core (cost model): DMA ~85us (28.3MB fp16 — roofline),
PE ~48us, DVE ~85-115us (864 small stts — watch this one), ACT ~3us.

Numerics: fp16 feat + fp16 w (no lo-correction) + fp16 p measured 2.0e-4
L2 rel err end-to-end in host simulation (gate is 2e-2).  Softmax runs
without a max pass: logits are bounded (~N(0,1.3)+[0,2]); constant bias -8
(folded into the host-side mask term) prevents overflow and cancels in v/s.
Host combines partials: out[b, c] = sum_q v / sum_q s (fp64).
"""
import sys

sys.path.insert(0, "/opt/trn_rl_repo")

import numpy as np

import concourse.bass as bass
import concourse.tile as tile
from concourse import mybir, bacc
from concourse.bass_utils import run_bass_kernel_spmd

B, C = 2, 128
N_FULL = 48 * 96 * 96          # 442368
N_CORES = 8
Q_PER_B = 4                    # spatial quarters per batch
NS = N_FULL // Q_PER_B         # 110592 per core
NT = NS // 128                 # 864 spatial tiles of 128
CH = 32                        # tiles per chunk
NCH = NT // CH                 # 27 chunks
EXP_BIAS = -8.0

f32 = mybir.dt.float32
f16 = mybir.dt.float16

_CACHED = {}


def _build(bench_reps=None, variant="full"):
    """bench_reps=None -> production straight-line kernel.
    bench_reps=R -> body wrapped in a For_i(R) repeat loop (for HW timing
    via wall-clock deltas between two R values).
    variant: ablation selector ("full", "dmaonly", "nostt", "nomm", "noexp")."""
    nc = bacc.Bacc("TRN2", target_bir_lowering=False, debug=False)

    featT_dram = nc.dram_tensor("featT", [128, NS], f16, kind="ExternalInput")
    maskT_dram = nc.dram_tensor("maskT", [128, NT], f32, kind="ExternalInput")
    wbc_dram = nc.dram_tensor("wbc", [128, 128], f16, kind="ExternalInput")
    vrow_dram = nc.dram_tensor("v_row", [1, 128], f32, kind="ExternalOutput")
    scols_dram = nc.dram_tensor("s_cols", [128, NCH], f32, kind="ExternalOutput")

    mult = mybir.AluOpType.mult
    add = mybir.AluOpType.add

    with tile.TileContext(nc) as tc:
        with (
            tc.tile_pool(name="weights", bufs=1) as wpool,
            tc.tile_pool(name="feat", bufs=4) as fpool,
            tc.tile_pool(name="junk", bufs=4) as jpool,
            tc.tile_pool(name="logit", bufs=3) as lpool,
            tc.tile_pool(name="prob", bufs=3) as ppool,
            tc.tile_pool(name="accs", bufs=1) as accpool,
            tc.tile_pool(name="psum", bufs=1, space="PSUM") as psum,
        ):
            wbc = wpool.tile([128, 128], f16)
            nc.sync.dma_start(wbc[:], wbc_dram.ap())
            maskT = wpool.tile([128, NT], f32)
            nc.sync.dma_start(maskT[:], maskT_dram.ap())

            s_cols = accpool.tile([128, NCH], f32)
            vps = psum.tile([1, 128], f32)
            if variant in ("dmaonly", "nomm", "noexp"):
                nc.vector.memset(s_cols[:], 1.0)

            def emit_chunk(ci):
                fT = fpool.tile([128, CH * 128], f16, tag="fT")
                nc.sync.dma_start(
                    fT[:], featT_dram.ap()[:, ci * CH * 128:(ci + 1) * CH * 128])
                if variant == "dmaonly":
                    return

                Lb = lpool.tile([128, CH], f32, tag="Lb")
                if variant == "nostt":
                    nc.vector.memset(Lb[:], 0.0)
                else:
                    for t in range(CH):
                        junk = jpool.tile([128, 128], f16, tag="junk")
                        nc.vector.scalar_tensor_tensor(
                            junk[:], fT[:, t * 128:(t + 1) * 128], 1.0, wbc[:],
                            op0=mult, op1=mult,
                            accum_out=Lb[:, t:t + 1],
                        )
                Lm = lpool.tile([128, CH], f32, tag="Lm")
                nc.vector.scalar_tensor_tensor(
                    Lm[:], Lb[:], 1.0, maskT[:, ci * CH:(ci + 1) * CH],
                    op0=mult, op1=add,
                )
                Pb = ppool.tile([128, CH], f16, tag="Pb")
                if variant != "noexp":
                    nc.scalar.activation(
                        Pb[:], Lm[:], mybir.ActivationFunctionType.Exp,
                        bias=0.0, scale=1.0,
                        accum_out=s_cols[:, ci:ci + 1],
                    )
                if variant != "nomm":
                    for t in range(CH):
                        gt = ci * CH + t
                        nc.tensor.matmul(
                            vps[:], Pb[:, t:t + 1], fT[:, t * 128:(t + 1) * 128],
                            start=(gt == 0), stop=(gt == NT - 1),
                        )

            def emit_all():
                for ci in range(NCH):
                    emit_chunk(ci)

            if bench_reps is None:
                emit_all()
            else:
                with tc.For_i(0, bench_reps, 1,
                              hint_engines=(mybir.EngineType.PE,)):
                    emit_all()

            v_sb = accpool.tile([1, 128], f32)
            if variant in ("dmaonly", "nomm"):
                nc.vector.memset(v_sb[:], 1.0)
            else:
                nc.scalar.copy(v_sb[:], vps[:])
            nc.sync.dma_start(vrow_dram.ap(), v_sb[:])
            nc.sync.dma_start(scols_dram.ap(), s_cols[:])

    nc.compile()
    return nc


def _get_nc(bench_reps=None, variant="full"):
    key = (bench_reps, variant)
    if key not in _CACHED:
        _CACHED[key] = _build(bench_reps, variant)
    return _CACHED[key]


def make_in_maps(feat, mask, w_attn):
    feat2 = np.asarray(feat).reshape(B, C, N_FULL)
    mask2 = 2.0 * np.clip(np.asarray(mask).reshape(B, N_FULL), 0.0, 1.0) + EXP_BIAS
    wh = np.asarray(w_attn).astype(np.float32).astype(np.float16)
    wbc = np.ascontiguousarray(np.tile(wh[None, :], (128, 1)))
    in_maps = []
    for core in range(N_CORES):
        b, q = divmod(core, Q_PER_B)
        shard = feat2[b, :, q * NS:(q + 1) * NS].astype(np.float16)  # [C, NS]
        fT = np.ascontiguousarray(
            shard.reshape(C, NT, 128).transpose(2, 1, 0).reshape(128, NS))
        mT = np.ascontiguousarray(
            mask2[b, q * NS:(q + 1) * NS].reshape(NT, 128).T.astype(np.float32))
        in_maps.append({
            "featT": fT,
            "maskT": mT,
            "wbc": wbc,
        })
    return in_maps


def combine(results):
    out = np.zeros((B, C), dtype=np.float32)
    for b in range(B):
        v = np.zeros(C, dtype=np.float64)
        s = 0.0
        for q in range(Q_PER_B):
            r = results[b * Q_PER_B + q]
            v += r["v_row"][0].astype(np.float64)
            s += float(r["s_cols"].astype(np.float64).sum())
        out[b] = (v / s).astype(np.float32)
    return out


def run_on_cores(feat, mask, w_attn, bench_reps=None):
    nc = _get_nc(bench_reps)
    in_maps = make_in_maps(np.asarray(feat), np.asarray(mask), np.asarray(w_attn))
    res = run_bass_kernel_spmd(nc, in_maps, core_ids=list(range(N_CORES)))
    return res


def kernel(feat, mask, w_attn):
    res = run_on_cores(feat, mask, w_attn)
    return combine(res.results)
